# revision 20
# baseline (speedup 1.0000x reference)
"""Trainium2 Bass kernel for nn_CascadedBranch_dynamic (CIF downsample -> proj ->
cosine-vs-token-table -> softmax(VQ) -> re-embed).

Sharding: vocab-parallel over the 8 cores. Each core holds V/8 = 6176 rows of the
token-embedding table (two layouts: D-major normalized for the score matmul,
V-major raw for the readout) and computes unnormalized softmax partials
u = sum_v exp(logit) * emb[v], s = sum_v exp(logit). Host combines u/s.
The audio/CIF path (~1.5% of FLOPs) is replicated on every core.

Matmuls run in fp16 (inputs cast host-side; PSUM accumulation is fp32).
|logits| <= 10 by construction (cosine / 0.1), so exp never overflows and no
max-subtraction pass is needed.
"""

import sys

for _p in ("/opt/trn_rl_repo", "/root/.axon_site/_ro/trn_rl_repo"):
    if _p not in sys.path:
        sys.path.insert(0, _p)

import numpy as np

import concourse.bacc as bacc
import concourse.bass as bass
import concourse.mybir as mybir
import concourse.tile as tile
from concourse import bass_utils

F16 = mybir.dt.float16
F32 = mybir.dt.float32
AF = mybir.ActivationFunctionType
OP = mybir.AluOpType

B, T, S, A, D, V = 8, 1024, 48, 768, 512, 49408
N_CORES = 8
VS = V // N_CORES            # 6176 rows of the table per core
NCH = (VS + 127) // 128      # 49 v-chunks (48 full + one of 32)
V_TAIL = VS - 128 * (NCH - 1)  # 32
TCH = T // 128               # 8 time chunks
AK = 7                       # 6 A-chunks of 128 + 1 aug chunk (bias row)
BS = B * S                   # 384 keyword rows
EPS = 1e-8
TEMP = 0.1

_CACHE = {}


def _build_program(debug=False, repeat=1):
    nc = bacc.Bacc("TRN2", target_bir_lowering=False, debug=False,
                   num_devices=N_CORES)

    audio_t = nc.dram_tensor("audio", [B, T, A], F16, kind="ExternalInput").ap()
    mt_t = nc.dram_tensor("mt", [128, TCH, B, S], F16, kind="ExternalInput").ap()
    waug_t = nc.dram_tensor("waug", [128, AK, D], F16, kind="ExternalInput").ap()
    ident_t = nc.dram_tensor("ident", [128, 128], F16, kind="ExternalInput").ap()
    ebnt_t = nc.dram_tensor("ebnt", [128, 4, VS], F16, kind="ExternalInput").ap()
    embn_t = nc.dram_tensor("embn", [128, NCH, D], F16, kind="ExternalInput").ap()
    u_t = nc.dram_tensor("u", [3, 128, D], F32, kind="ExternalOutput").ap()
    s_t = nc.dram_tensor("s", [3, 128, 1], F32, kind="ExternalOutput").ap()
    if debug:
        dbg_kwnt_t = nc.dram_tensor("dbg_kwnt", [128, 4, BS], F16,
                                    kind="ExternalOutput").ap()
        dbg_pt_t = nc.dram_tensor("dbg_pt", [128, BS], F16,
                                  kind="ExternalOutput").ap()
        dbg_ds_t = nc.dram_tensor("dbg_ds", [B, S, A], F16,
                                  kind="ExternalOutput").ap()

    with tile.TileContext(nc) as tc:
        with (
            tc.tile_pool(name="const", bufs=1) as const,
            tc.tile_pool(name="audio", bufs=2) as audio_pool,
            tc.tile_pool(name="ds", bufs=2) as ds_pool,
            tc.tile_pool(name="dst", bufs=2) as dst_pool,
            tc.tile_pool(name="kwn", bufs=2) as kwn_pool,
            tc.tile_pool(name="sq", bufs=2) as sq_pool,
            tc.tile_pool(name="pt", bufs=3) as pt_pool,
            tc.tile_pool(name="outp", bufs=2) as out_pool,
        ):

            def body():
                # ---- resident tables / constants ----
                mt_sb = const.tile([128, TCH, B, S], F16, name="mt_sb", tag="mt")
                nc.sync.dma_start(mt_sb[:], mt_t[:])
                waug_sb = const.tile([128, AK, D], F16, name="waug_sb", tag="wa")
                nc.sync.dma_start(waug_sb[:], waug_t[:])
                ident_sb = const.tile([128, 128], F16, name="ident_sb", tag="id")
                nc.sync.dma_start(ident_sb[:], ident_t[:])
                ones_sb = const.tile([128, 1], F16, name="ones_sb", tag="on")
                nc.vector.memset(ones_sb[:], 1.0)

                ebnt_sb = const.tile([128, 4, VS], F16, name="ebnt_sb", tag="et")
                for k in range(4):
                    nc.scalar.dma_start(ebnt_sb[:, k, :], ebnt_t[:, k, :])
                embn_sb = const.tile([128, NCH, D], F16, name="embn_sb", tag="en")
                for g in range(4):
                    c0 = g * 13
                    c1 = min(NCH, c0 + 13)
                    nc.scalar.dma_start(embn_sb[:, c0:c1, :], embn_t[:, c0:c1, :])

                kwnT_sb = const.tile([128, 4, BS], F16, name="kwnT_sb", tag="kt")
                kn2_sb = const.tile([S, B], F32, name="kn2_sb", tag="k2")
                rf_sb = const.tile([S, B], F32, name="rf_sb", tag="rf")
                qq_sb = const.tile([S, B], F32, name="qq_sb", tag="qq")
                nt_sb = const.tile([S, B], F32, name="nt_sb", tag="nt")

                # ---- audio/CIF path, per batch ----
                prelude_cm = tc.tile_pool(name="preps", bufs=1,
                                          space=bass.MemorySpace.PSUM)
                t_cm = tc.tile_pool(name="tps", bufs=2,
                                    space=bass.MemorySpace.PSUM)
                ds_ps = kw_ps = prelude_cm.__enter__()
                t_ps = t_cm.__enter__()
                for b in range(B):
                    at = audio_pool.tile([128, TCH, A], F16, name="at", tag="at")
                    nc.sync.dma_start(
                        at[:], audio_t[b].rearrange("(c p) a -> p c a", p=128))

                    dsp0 = ds_ps.tile([S, 384], F32, name="dsp0", tag="dsp0")
                    dsp1 = ds_ps.tile([S, 384], F32, name="dsp1", tag="dsp1")
                    for c in range(TCH):
                        nc.tensor.matmul(dsp0[:], mt_sb[:, c, b, :],
                                         at[:, c, 0:384],
                                         start=(c == 0), stop=(c == TCH - 1))
                        nc.tensor.matmul(dsp1[:], mt_sb[:, c, b, :],
                                         at[:, c, 384:768],
                                         start=(c == 0), stop=(c == TCH - 1))

                    ds_sb = ds_pool.tile([S, A], F16, name="ds_sb", tag="ds")
                    nc.vector.tensor_copy(ds_sb[:, 0:384], dsp0[:])
                    nc.vector.tensor_copy(ds_sb[:, 384:768], dsp1[:])
                    if debug:
                        nc.sync.dma_start(dbg_ds_t[b], ds_sb[:])

                    dsT_sb = dst_pool.tile([128, AK, S], F16, name="dsT_sb",
                                           tag="dst")
                    nc.vector.memset(dsT_sb[:, 6, :], 0.0)
                    nc.vector.memset(dsT_sb[0:1, 6, :], 1.0)
                    for k in range(6):
                        tp = t_ps.tile([128, S], F16, name="tp", tag="tp")
                        nc.tensor.transpose(
                            tp[:], ds_sb[:, k * 128:(k + 1) * 128],
                            ident_sb[:S, :S])
                        nc.vector.tensor_copy(dsT_sb[:, k, :], tp[:])

                    kwp = kw_ps.tile([S, D], F32, name="kwp", tag="kwp")
                    for k in range(AK):
                        nc.tensor.matmul(kwp[:], dsT_sb[:, k, :],
                                         waug_sb[:, k, :],
                                         start=(k == 0), stop=(k == AK - 1))

                    # rf = 10 / max(||kw||, 1e-8)  ==  rsqrt(kn2 * 0.01)
                    sq_sb = sq_pool.tile([S, D], F16, name="sq_sb", tag="sq")
                    nc.scalar.activation(sq_sb[:], kwp[:], AF.Square,
                                         accum_out=kn2_sb[:, b:b + 1])
                    q = qq_sb[:, b:b + 1]
                    nc.scalar.mul(q, kn2_sb[:, b:b + 1], float(TEMP * TEMP))
                    y = rf_sb[:, b:b + 1]
                    nc.scalar.activation(y, q, AF.Sqrt)
                    nc.vector.tensor_scalar_max(y, y, EPS * TEMP)
                    nc.vector.reciprocal(y, y)
                    t1 = nt_sb[:, b:b + 1]
                    for _ in range(2):
                        nc.vector.tensor_tensor(t1, y, y, OP.mult)
                        nc.vector.tensor_tensor(t1, t1, q, OP.mult)
                        nc.vector.tensor_scalar(t1, t1, -0.5, 1.5,
                                                OP.mult, OP.add)
                        nc.vector.tensor_tensor(y, y, t1, OP.mult)

                    kwn_sb = kwn_pool.tile([S, D], F16, name="kwn_sb", tag="kw")
                    nc.vector.tensor_scalar_mul(kwn_sb[:], kwp[:], y)
                    for k in range(4):
                        tp = t_ps.tile([128, S], F16, name="tp", tag="tp")
                        nc.tensor.transpose(
                            tp[:], kwn_sb[:, k * 128:(k + 1) * 128],
                            ident_sb[:S, :S])
                        nc.vector.tensor_copy(kwnT_sb[:, k, b * S:(b + 1) * S],
                                              tp[:])

                t_cm.__exit__(None, None, None)
                prelude_cm.__exit__(None, None, None)

                # ---- vocab chunk loop ----
                p_cm = tc.tile_pool(name="pps", bufs=2,
                                    space=bass.MemorySpace.PSUM)
                u_cm = tc.tile_pool(name="ups", bufs=1,
                                    space=bass.MemorySpace.PSUM)
                s_cm = tc.tile_pool(name="sps", bufs=1,
                                    space=bass.MemorySpace.PSUM)
                p_ps = p_cm.__enter__()
                u_psp = u_cm.__enter__()
                s_psp = s_cm.__enter__()
                u_tiles = [u_psp.tile([128, D], F32, tag=f"u{i}", name=f"u{i}")
                           for i in range(3)]
                s_tiles = [s_psp.tile([128, 1], F32, tag=f"s{i}", name=f"s{i}")
                           for i in range(3)]
                for c in range(NCH):
                    vsz = 128 if c < NCH - 1 else V_TAIL
                    pp = p_ps.tile([128, BS], F32, name="pp", tag="pp")
                    for k in range(4):
                        nc.tensor.matmul(
                            pp[:vsz, :], ebnt_sb[:, k, c * 128:c * 128 + vsz],
                            kwnT_sb[:, k, :], start=(k == 0), stop=(k == 3))
                    pt = pt_pool.tile([128, BS], F16, name="pt", tag="pt")
                    nc.scalar.activation(pt[:vsz, :], pp[:vsz, :], AF.Exp)
                    if debug and c == 0:
                        nc.sync.dma_start(dbg_kwnt_t[:], kwnT_sb[:])
                        nc.sync.dma_start(dbg_pt_t[:], pt[:])
                    for blk in range(3):
                        nc.tensor.matmul(
                            u_tiles[blk][:], pt[:vsz, blk * 128:(blk + 1) * 128],
                            embn_sb[:vsz, c, :],
                            start=(c == 0), stop=(c == NCH - 1))
                        nc.tensor.matmul(
                            s_tiles[blk][:],
                            pt[:vsz, blk * 128:(blk + 1) * 128],
                            ones_sb[:vsz, :],
                            start=(c == 0), stop=(c == NCH - 1))

                # ---- write out partials ----
                for blk in range(3):
                    us = out_pool.tile([128, D], F32, name="us", tag="us")
                    nc.vector.tensor_copy(us[:], u_tiles[blk][:])
                    nc.sync.dma_start(u_t[blk], us[:])
                    ss = out_pool.tile([128, 1], F32, name="ss", tag="ss")
                    nc.vector.tensor_copy(ss[:], s_tiles[blk][:])
                    nc.sync.dma_start(s_t[blk], ss[:])

                s_cm.__exit__(None, None, None)
                u_cm.__exit__(None, None, None)
                p_cm.__exit__(None, None, None)

            if repeat == 1:
                body()
            else:
                with tc.For_i(0, repeat, 1):
                    body()

    nc.compile()
    return nc


def _host_prep(audio_feat, W_proj, b_proj, token_emb, fp_alignment):
    """Build the per-core input maps (dtype casts, layout shuffles, CIF alpha)."""
    audio16 = np.ascontiguousarray(audio_feat.astype(np.float16))

    # CIF pseudo-alpha matrix M_T[t, b, s] = 1/len_s on frames of segment s.
    fp = np.clip(fp_alignment.astype(np.int64), 0, T)          # [B, S] == cumsum
    lens = np.diff(fp, prepend=0, axis=-1)
    lens = np.clip(lens, 0, None)
    cum = np.cumsum(lens, axis=-1)                             # [B, S]
    start = cum - lens
    tidx = np.arange(T)
    ind = (tidx[None, :, None] >= start[:, None, :]) & \
          (tidx[None, :, None] < cum[:, None, :])              # [B, T, S]
    recip = np.where(lens > 0, 1.0 / np.maximum(lens, 1), 0.0) # [B, S]
    mt = ind * recip[:, None, :]                               # [B, T, S]
    mt16 = np.ascontiguousarray(
        mt.reshape(B, TCH, 128, S).transpose(2, 1, 0, 3).astype(np.float16))

    waug = np.zeros((AK * 128, D), np.float32)
    waug[:A] = W_proj
    waug[A] = b_proj
    waug16 = np.ascontiguousarray(
        waug.reshape(AK, 128, D).transpose(1, 0, 2).astype(np.float16))

    ident16 = np.eye(128, dtype=np.float16)

    en = np.maximum(np.linalg.norm(token_emb, axis=-1), EPS)   # [V]
    ebn = token_emb / en[:, None]                              # [V, D] f32

    shared = {"audio": audio16, "mt": mt16, "waug": waug16, "ident": ident16}
    in_maps = []
    for i in range(N_CORES):
        v0 = i * VS
        sl_n = ebn[v0:v0 + VS]                                 # [VS, D]
        ebnt16 = np.ascontiguousarray(
            sl_n.T.reshape(4, 128, VS).transpose(1, 0, 2).astype(np.float16))
        raw = np.zeros((NCH * 128, D), np.float32)
        raw[:VS] = token_emb[v0:v0 + VS]
        embn16 = np.ascontiguousarray(
            raw.reshape(NCH, 128, D).transpose(1, 0, 2).astype(np.float16))
        in_maps.append({**shared, "ebnt": ebnt16, "embn": embn16})
    return in_maps


def kernel(audio_feat, W_proj, b_proj, token_emb, fp_alignment, target_len):
    assert int(target_len) == S
    if "nc" not in _CACHE:
        _CACHE["nc"] = _build_program()
    nc = _CACHE["nc"]

    in_maps = _host_prep(np.asarray(audio_feat, np.float32),
                         np.asarray(W_proj, np.float32),
                         np.asarray(b_proj, np.float32),
                         np.asarray(token_emb, np.float32),
                         np.asarray(fp_alignment))

    res = bass_utils.run_bass_kernel_spmd(nc, in_maps,
                                          core_ids=list(range(N_CORES)))
    u = np.zeros((3, 128, D), np.float64)
    s = np.zeros((3, 128, 1), np.float64)
    for i in range(N_CORES):
        u += res.results[i]["u"]
        s += res.results[i]["s"]
    out = (u.reshape(BS, D) / s.reshape(BS, 1)).astype(np.float32)
    return out.reshape(B, S, D)


# revision 25
# speedup vs baseline: 1.2279x; 1.2279x over previous
"""Trainium2 Bass kernel for nn_CascadedBranch_dynamic (CIF downsample -> proj ->
cosine-vs-token-table -> softmax(VQ) -> re-embed).

Sharding: vocab-parallel over the 8 cores. Each core holds V/8 = 6176 rows of the
token-embedding table (two layouts: D-major normalized for the score matmul,
V-major raw for the readout) and computes unnormalized softmax partials
u = sum_v exp(logit) * emb[v], s = sum_v exp(logit). Host combines u/s.
The audio/CIF path (~1.5% of FLOPs) is replicated on every core.

Matmuls run in fp16 (inputs cast host-side; PSUM accumulation is fp32).
|logits| <= 10 by construction (cosine / 0.1), so exp never overflows and no
max-subtraction pass is needed.
"""

import sys

for _p in ("/opt/trn_rl_repo", "/root/.axon_site/_ro/trn_rl_repo"):
    if _p not in sys.path:
        sys.path.insert(0, _p)

import numpy as np

import concourse.bacc as bacc
import concourse.bass as bass
import concourse.mybir as mybir
import concourse.tile as tile
from concourse import bass_utils

F16 = mybir.dt.float16
F32 = mybir.dt.float32
AF = mybir.ActivationFunctionType
OP = mybir.AluOpType

B, T, S, A, D, V = 8, 1024, 48, 768, 512, 49408
N_CORES = 8
VS = V // N_CORES            # 6176 rows of the table per core
NCH = (VS + 127) // 128      # 49 v-chunks (48 full + one of 32)
V_TAIL = VS - 128 * (NCH - 1)  # 32
TCH = T // 128               # 8 time chunks
AK = 7                       # 6 A-chunks of 128 + 1 aug chunk (bias row)
BS = B * S                   # 384 keyword rows
EPS = 1e-8
TEMP = 0.1
VGRP = 7                     # v-chunks per table-DMA group
NGRP = (NCH + VGRP - 1) // VGRP  # 7 groups

_CACHE = {}


def _build_program(debug=False, repeat=1):
    nc = bacc.Bacc("TRN2", target_bir_lowering=False, debug=False,
                   num_devices=N_CORES)

    # audio pre-laid-out host-side: [B, 128, TCH, A] contiguous per partition
    audio_t = nc.dram_tensor("audio", [B, 128, TCH, A], F16,
                             kind="ExternalInput").ap()
    mt_t = nc.dram_tensor("mt", [128, TCH, B, S], F16, kind="ExternalInput").ap()
    waug_t = nc.dram_tensor("waug", [128, AK, D], F16, kind="ExternalInput").ap()
    ident_t = nc.dram_tensor("ident", [128, 128], F16, kind="ExternalInput").ap()
    # per-group tables: ebnt [128, NGRP, 4, VGRP*128], embn [128, NGRP, VGRP, D]
    ebnt_t = nc.dram_tensor("ebnt", [128, NGRP, 4, VGRP * 128], F16,
                            kind="ExternalInput").ap()
    embn_t = nc.dram_tensor("embn", [128, NGRP, VGRP, D], F16,
                            kind="ExternalInput").ap()
    u_t = nc.dram_tensor("u", [3, 128, D], F32, kind="ExternalOutput").ap()
    s_t = nc.dram_tensor("s", [3, 128, 1], F32, kind="ExternalOutput").ap()
    if debug:
        dbg_kwnt_t = nc.dram_tensor("dbg_kwnt", [128, 4, BS], F16,
                                    kind="ExternalOutput").ap()
        dbg_pt_t = nc.dram_tensor("dbg_pt", [128, BS], F16,
                                  kind="ExternalOutput").ap()
        dbg_ds_t = nc.dram_tensor("dbg_ds", [B, S, A], F16,
                                  kind="ExternalOutput").ap()

    with tile.TileContext(nc) as tc:
        with (
            tc.tile_pool(name="const", bufs=1) as const,
            tc.tile_pool(name="audio", bufs=3) as audio_pool,
            tc.tile_pool(name="ds", bufs=2) as ds_pool,
            tc.tile_pool(name="dst", bufs=2) as dst_pool,
            tc.tile_pool(name="kwn", bufs=2) as kwn_pool,
            tc.tile_pool(name="sq", bufs=2) as sq_pool,
            tc.tile_pool(name="pt", bufs=4) as pt_pool,
            tc.tile_pool(name="outp", bufs=2) as out_pool,
        ):

            def body():
                # ---- tiny constants first (cheap DMAs) ----
                mt_sb = const.tile([128, TCH, B, S], F16, name="mt_sb", tag="mt")
                nc.sync.dma_start(mt_sb[:], mt_t[:])
                waug_sb = const.tile([128, AK, D], F16, name="waug_sb", tag="wa")
                nc.sync.dma_start(waug_sb[:], waug_t[:])
                ident_sb = const.tile([128, 128], F16, name="ident_sb", tag="id")
                nc.sync.dma_start(ident_sb[:], ident_t[:])
                ones_sb = const.tile([128, 1], F16, name="ones_sb", tag="on")
                nc.vector.memset(ones_sb[:], 1.0)

                # ---- audio loads first: they gate the whole prelude ----
                at_tiles = []
                for b in range(B):
                    at = audio_pool.tile([128, TCH, A], F16, name="at",
                                         tag="at")
                    nc.sync.dma_start(at[:], audio_t[b])
                    at_tiles.append(at)

                # ---- table group loads, in consumption order ----
                ebnt_g, embn_g = [], []
                for g in range(NGRP):
                    eg = const.tile([128, 4, VGRP * 128], F16,
                                    name=f"ebnt{g}", tag=f"eb{g}")
                    nc.sync.dma_start(eg[:], ebnt_t[:, g])
                    ng = const.tile([128, VGRP, D], F16,
                                    name=f"embn{g}", tag=f"em{g}")
                    nc.sync.dma_start(ng[:], embn_t[:, g])
                    ebnt_g.append(eg)
                    embn_g.append(ng)

                kwnT_sb = const.tile([128, 4, BS], F16, name="kwnT_sb", tag="kt")
                kn2_sb = const.tile([S, B], F32, name="kn2_sb", tag="k2")
                rf_sb = const.tile([S, B], F32, name="rf_sb", tag="rf")
                qq_sb = const.tile([S, B], F32, name="qq_sb", tag="qq")
                nt_sb = const.tile([S, B], F32, name="nt_sb", tag="nt")

                # ---- audio/CIF path, per batch ----
                prelude_cm = tc.tile_pool(name="preps", bufs=1,
                                          space=bass.MemorySpace.PSUM)
                t_cm = tc.tile_pool(name="tps", bufs=2,
                                    space=bass.MemorySpace.PSUM)
                ds_ps = kw_ps = prelude_cm.__enter__()
                t_ps = t_cm.__enter__()
                for b in range(B):
                    at = at_tiles[b]
                    dsp0 = ds_ps.tile([S, 384], F32, name="dsp0", tag="dsp0")
                    dsp1 = ds_ps.tile([S, 384], F32, name="dsp1", tag="dsp1")
                    for c in range(TCH):
                        nc.tensor.matmul(dsp0[:], mt_sb[:, c, b, :],
                                         at[:, c, 0:384],
                                         start=(c == 0), stop=(c == TCH - 1))
                        nc.tensor.matmul(dsp1[:], mt_sb[:, c, b, :],
                                         at[:, c, 384:768],
                                         start=(c == 0), stop=(c == TCH - 1))

                    ds_sb = ds_pool.tile([S, A], F16, name="ds_sb", tag="ds")
                    nc.vector.tensor_copy(ds_sb[:, 0:384], dsp0[:])
                    nc.vector.tensor_copy(ds_sb[:, 384:768], dsp1[:])
                    if debug:
                        nc.sync.dma_start(dbg_ds_t[b], ds_sb[:])

                    dsT_sb = dst_pool.tile([128, AK, S], F16, name="dsT_sb",
                                           tag="dst")
                    nc.vector.memset(dsT_sb[:, 6, :], 0.0)
                    nc.vector.memset(dsT_sb[0:1, 6, :], 1.0)
                    for k in range(6):
                        tp = t_ps.tile([128, S], F16, name="tp", tag="tp")
                        nc.tensor.transpose(
                            tp[:], ds_sb[:, k * 128:(k + 1) * 128],
                            ident_sb[:S, :S])
                        nc.vector.tensor_copy(dsT_sb[:, k, :], tp[:])

                    kwp = kw_ps.tile([S, D], F32, name="kwp", tag="kwp")
                    for k in range(AK):
                        nc.tensor.matmul(kwp[:], dsT_sb[:, k, :],
                                         waug_sb[:, k, :],
                                         start=(k == 0), stop=(k == AK - 1))

                    # rf = 10 / max(||kw||, 1e-8)  ==  rsqrt(kn2 * 0.01)
                    sq_sb = sq_pool.tile([S, D], F16, name="sq_sb", tag="sq")
                    nc.scalar.activation(sq_sb[:], kwp[:], AF.Square,
                                         accum_out=kn2_sb[:, b:b + 1])
                    q = qq_sb[:, b:b + 1]
                    nc.scalar.mul(q, kn2_sb[:, b:b + 1], float(TEMP * TEMP))
                    y = rf_sb[:, b:b + 1]
                    nc.scalar.activation(y, q, AF.Sqrt)
                    nc.vector.tensor_scalar_max(y, y, EPS * TEMP)
                    nc.vector.reciprocal(y, y)
                    t1 = nt_sb[:, b:b + 1]
                    for _ in range(2):
                        nc.vector.tensor_tensor(t1, y, y, OP.mult)
                        nc.vector.tensor_tensor(t1, t1, q, OP.mult)
                        nc.vector.tensor_scalar(t1, t1, -0.5, 1.5,
                                                OP.mult, OP.add)
                        nc.vector.tensor_tensor(y, y, t1, OP.mult)

                    kwn_sb = kwn_pool.tile([S, D], F16, name="kwn_sb", tag="kw")
                    nc.vector.tensor_scalar_mul(kwn_sb[:], kwp[:], y)
                    for k in range(4):
                        tp = t_ps.tile([128, S], F16, name="tp", tag="tp")
                        nc.tensor.transpose(
                            tp[:], kwn_sb[:, k * 128:(k + 1) * 128],
                            ident_sb[:S, :S])
                        nc.vector.tensor_copy(kwnT_sb[:, k, b * S:(b + 1) * S],
                                              tp[:])

                t_cm.__exit__(None, None, None)
                prelude_cm.__exit__(None, None, None)

                # ---- vocab chunk loop: super-chunks of 2 v-chunks ----
                p_cm = tc.tile_pool(name="pps", bufs=2,
                                    space=bass.MemorySpace.PSUM)
                u_cm = tc.tile_pool(name="ups", bufs=1,
                                    space=bass.MemorySpace.PSUM)
                s_cm = tc.tile_pool(name="sps", bufs=1,
                                    space=bass.MemorySpace.PSUM)
                p_ps = p_cm.__enter__()
                u_psp = u_cm.__enter__()
                s_psp = s_cm.__enter__()
                u_tiles = [u_psp.tile([128, D], F32, tag=f"u{i}", name=f"u{i}")
                           for i in range(3)]
                s_tile = s_psp.tile([128, 3], F32, tag="s", name="s")
                nsc = (NCH + 1) // 2
                for sc in range(nsc):
                    chunks = [c for c in (2 * sc, 2 * sc + 1) if c < NCH]
                    # pp2: [128, 2, 512] f32 = exactly 2 PSUM banks
                    pp = p_ps.tile([128, 2, D], F32, name="pp", tag="pp")
                    for gi, c in enumerate(chunks):
                        vsz = 128 if c < NCH - 1 else V_TAIL
                        g, off = divmod(c, VGRP)
                        for k in range(4):
                            nc.tensor.matmul(
                                pp[:vsz, gi, 0:BS],
                                ebnt_g[g][:, k, off * 128:off * 128 + vsz],
                                kwnT_sb[:, k, :], start=(k == 0), stop=(k == 3))
                    pt = pt_pool.tile([128, 2, BS], F16, name="pt", tag="pt")
                    if len(chunks) == 2:
                        nc.scalar.activation(pt[:, :, :], pp[:, :, 0:BS], AF.Exp)
                    else:
                        vsz = V_TAIL
                        nc.scalar.activation(pt[:vsz, 0, :], pp[:vsz, 0, 0:BS],
                                             AF.Exp)
                    if debug and sc == 0:
                        nc.sync.dma_start(dbg_kwnt_t[:], kwnT_sb[:])
                        nc.sync.dma_start(dbg_pt_t[:], pt[:, 0, :])
                    for gi, c in enumerate(chunks):
                        vsz = 128 if c < NCH - 1 else V_TAIL
                        g, off = divmod(c, VGRP)
                        first = c == 0
                        last = c == NCH - 1
                        for blk in range(3):
                            nc.tensor.matmul(
                                u_tiles[blk][:],
                                pt[:vsz, gi, blk * 128:(blk + 1) * 128],
                                embn_g[g][:vsz, off, :],
                                start=first, stop=last)
                            # all three share one PSUM bank: the first matmul's
                            # start=True clears the whole bank, the others
                            # accumulate onto the cleared lanes.
                            nc.tensor.matmul(
                                s_tile[:, blk:blk + 1],
                                pt[:vsz, gi, blk * 128:(blk + 1) * 128],
                                ones_sb[:vsz, :],
                                start=(first and blk == 0), stop=last)

                # ---- write out partials ----
                for blk in range(3):
                    us = out_pool.tile([128, D], F32, name="us", tag="us")
                    nc.vector.tensor_copy(us[:], u_tiles[blk][:])
                    nc.sync.dma_start(u_t[blk], us[:])
                    ss = out_pool.tile([128, 1], F32, name="ss", tag="ss")
                    nc.vector.tensor_copy(ss[:], s_tile[:, blk:blk + 1])
                    nc.sync.dma_start(s_t[blk], ss[:])

                s_cm.__exit__(None, None, None)
                u_cm.__exit__(None, None, None)
                p_cm.__exit__(None, None, None)

            if repeat == 1:
                body()
            else:
                with tc.For_i(0, repeat, 1):
                    body()

    nc.compile()
    return nc


def _host_prep(audio_feat, W_proj, b_proj, token_emb, fp_alignment):
    """Build the per-core input maps (dtype casts, layout shuffles, CIF alpha)."""
    audio16 = np.ascontiguousarray(
        audio_feat.astype(np.float16)
        .reshape(B, TCH, 128, A).transpose(0, 2, 1, 3))

    # CIF pseudo-alpha matrix M_T[t, b, s] = 1/len_s on frames of segment s.
    fp = np.clip(fp_alignment.astype(np.int64), 0, T)          # [B, S] == cumsum
    lens = np.diff(fp, prepend=0, axis=-1)
    lens = np.clip(lens, 0, None)
    cum = np.cumsum(lens, axis=-1)                             # [B, S]
    start = cum - lens
    tidx = np.arange(T)
    ind = (tidx[None, :, None] >= start[:, None, :]) & \
          (tidx[None, :, None] < cum[:, None, :])              # [B, T, S]
    recip = np.where(lens > 0, 1.0 / np.maximum(lens, 1), 0.0) # [B, S]
    mt = ind * recip[:, None, :]                               # [B, T, S]
    mt16 = np.ascontiguousarray(
        mt.reshape(B, TCH, 128, S).transpose(2, 1, 0, 3).astype(np.float16))

    waug = np.zeros((AK * 128, D), np.float32)
    waug[:A] = W_proj
    waug[A] = b_proj
    waug16 = np.ascontiguousarray(
        waug.reshape(AK, 128, D).transpose(1, 0, 2).astype(np.float16))

    ident16 = np.eye(128, dtype=np.float16)

    en = np.maximum(np.linalg.norm(token_emb, axis=-1), EPS)   # [V]
    ebn = token_emb / en[:, None]                              # [V, D] f32

    shared = {"audio": audio16, "mt": mt16, "waug": waug16, "ident": ident16}
    in_maps = []
    NV = NGRP * VGRP * 128                                     # padded row count
    for i in range(N_CORES):
        v0 = i * VS
        sl_n = np.zeros((NV, D), np.float32)
        sl_n[:VS] = ebn[v0:v0 + VS]
        # ebnt: [128p(of D), NGRP, 4, VGRP*128] ; D index = k*128 + p
        et = sl_n.T.reshape(4, 128, NGRP, VGRP * 128)
        ebnt16 = np.ascontiguousarray(
            et.transpose(1, 2, 0, 3).astype(np.float16))
        raw = np.zeros((NV, D), np.float32)
        raw[:VS] = token_emb[v0:v0 + VS]
        # embn: [128p(of v), NGRP, VGRP, D] ; v = (g*VGRP + j)*128 + p
        embn16 = np.ascontiguousarray(
            raw.reshape(NGRP, VGRP, 128, D).transpose(2, 0, 1, 3)
            .astype(np.float16))
        in_maps.append({**shared, "ebnt": ebnt16, "embn": embn16})
    return in_maps


def kernel(audio_feat, W_proj, b_proj, token_emb, fp_alignment, target_len):
    assert int(target_len) == S
    if "nc" not in _CACHE:
        _CACHE["nc"] = _build_program()
    nc = _CACHE["nc"]

    in_maps = _host_prep(np.asarray(audio_feat, np.float32),
                         np.asarray(W_proj, np.float32),
                         np.asarray(b_proj, np.float32),
                         np.asarray(token_emb, np.float32),
                         np.asarray(fp_alignment))

    res = bass_utils.run_bass_kernel_spmd(nc, in_maps,
                                          core_ids=list(range(N_CORES)))
    u = np.zeros((3, 128, D), np.float64)
    s = np.zeros((3, 128, 1), np.float64)
    for i in range(N_CORES):
        u += res.results[i]["u"]
        s += res.results[i]["s"]
    out = (u.reshape(BS, D) / s.reshape(BS, 1)).astype(np.float32)
    return out.reshape(B, S, D)


# revision 31
# speedup vs baseline: 1.3810x; 1.1247x over previous
"""Trainium2 Bass kernel for nn_CascadedBranch_dynamic (CIF downsample -> proj ->
cosine-vs-token-table -> softmax(VQ) -> re-embed).

Sharding: vocab-parallel over the 8 cores. Each core holds V/8 = 6176 rows of the
token-embedding table (two layouts: D-major normalized for the score matmul,
V-major raw for the readout) and computes unnormalized softmax partials
u = sum_v exp(logit) * emb[v], s = sum_v exp(logit). Host combines u/s.
The audio/CIF path (~1.5% of FLOPs) is replicated on every core, processed two
batches at a time packed into disjoint PE column groups.

Matmuls run in fp16 (inputs cast host-side; PSUM accumulation is fp32).
|logits| <= 10 by construction (cosine / 0.1), so exp never overflows and no
max-subtraction pass is needed.
"""

import sys

for _p in ("/opt/trn_rl_repo", "/root/.axon_site/_ro/trn_rl_repo"):
    if _p not in sys.path:
        sys.path.insert(0, _p)

import numpy as np

import concourse.bacc as bacc
import concourse.bass as bass
import concourse.mybir as mybir
import concourse.tile as tile
from concourse import bass_utils

F16 = mybir.dt.float16
F32 = mybir.dt.float32
AF = mybir.ActivationFunctionType
OP = mybir.AluOpType

B, T, S, A, D, V = 8, 1024, 48, 768, 512, 49408
N_CORES = 8
VS = V // N_CORES            # 6176 rows of the table per core
NCH = (VS + 127) // 128      # 49 v-chunks (48 full + one of 32)
V_TAIL = VS - 128 * (NCH - 1)  # 32
TCH = T // 128               # 8 time chunks
AK = 7                       # 6 A-chunks of 128 + 1 aug chunk (bias row)
BS = B * S                   # 384 keyword rows
S2 = 2 * S                   # 96 keyword rows per batch pair
SP = 112                     # pair tiles: rows 0-47 = b0, 64-111 = b1
EPS = 1e-8
TEMP = 0.1
VGRP = 7                     # v-chunks per table-DMA group
NGRP = (NCH + VGRP - 1) // VGRP  # 7 groups

_CACHE = {}


def _build_program(debug=False, repeat=1):
    nc = bacc.Bacc("TRN2", target_bir_lowering=False, debug=False,
                   num_devices=N_CORES)

    # audio pre-laid-out host-side: [B, 128, TCH, A] contiguous per partition
    audio_t = nc.dram_tensor("audio", [B, 128, TCH, A], F16,
                             kind="ExternalInput").ap()
    mt_t = nc.dram_tensor("mt", [128, TCH, B, S], F16, kind="ExternalInput").ap()
    waug_t = nc.dram_tensor("waug", [128, AK, D], F16, kind="ExternalInput").ap()
    ident_t = nc.dram_tensor("ident", [128, 128], F16, kind="ExternalInput").ap()
    # per-group tables: ebnt [128, NGRP, 4, VGRP*128], embn [128, NGRP, VGRP, D]
    ebnt_t = nc.dram_tensor("ebnt", [128, NGRP, 4, VGRP * 128], F16,
                            kind="ExternalInput").ap()
    embn_t = nc.dram_tensor("embn", [128, NGRP, VGRP, D], F16,
                            kind="ExternalInput").ap()
    u_t = nc.dram_tensor("u", [3, 128, D], F32, kind="ExternalOutput").ap()
    s_t = nc.dram_tensor("s", [1, BS], F32, kind="ExternalOutput").ap()
    if debug:
        dbg_kwnt_t = nc.dram_tensor("dbg_kwnt", [128, 4, BS], F16,
                                    kind="ExternalOutput").ap()
        dbg_pt_t = nc.dram_tensor("dbg_pt", [128, BS], F16,
                                  kind="ExternalOutput").ap()

    with tile.TileContext(nc) as tc:
        with (
            tc.tile_pool(name="const", bufs=1) as const,
            tc.tile_pool(name="audio", bufs=3) as audio_pool,
            tc.tile_pool(name="ds", bufs=2) as ds_pool,
            tc.tile_pool(name="dst", bufs=2) as dst_pool,
            tc.tile_pool(name="kwn", bufs=2) as kwn_pool,
            tc.tile_pool(name="sq", bufs=2) as sq_pool,
            tc.tile_pool(name="pt", bufs=4) as pt_pool,
            tc.tile_pool(name="outp", bufs=2) as out_pool,
        ):

            def body():
                # ---- tiny constants first (cheap DMAs) ----
                mt_sb = const.tile([128, TCH, B, S], F16, name="mt_sb", tag="mt")
                nc.sync.dma_start(mt_sb[:], mt_t[:])
                waug_sb = const.tile([128, AK, D], F16, name="waug_sb", tag="wa")
                nc.sync.dma_start(waug_sb[:], waug_t[:])
                ident_sb = const.tile([128, 128], F16, name="ident_sb", tag="id")
                nc.sync.dma_start(ident_sb[:], ident_t[:])
                ones_sb = const.tile([128, 1], F16, name="ones_sb", tag="on")
                nc.vector.memset(ones_sb[:], 1.0)

                # ---- audio loads first: they gate the whole prelude ----
                at_tiles = []
                for b in range(B):
                    at = audio_pool.tile([128, TCH, A], F16, name="at",
                                         tag="at")
                    nc.sync.dma_start(at[:], audio_t[b])
                    at_tiles.append(at)

                # ---- table group loads, in consumption order ----
                ebnt_g, embn_g = [], []
                for g in range(NGRP):
                    eg = const.tile([128, 4, VGRP * 128], F16,
                                    name=f"ebnt{g}", tag=f"eb{g}")
                    nc.sync.dma_start(eg[:], ebnt_t[:, g])
                    ng = const.tile([128, VGRP, D], F16,
                                    name=f"embn{g}", tag=f"em{g}")
                    nc.sync.dma_start(ng[:], embn_t[:, g])
                    ebnt_g.append(eg)
                    embn_g.append(ng)

                kwnT_sb = const.tile([128, 4, BS], F16, name="kwnT_sb", tag="kt")
                kn2_sb = const.tile([SP, 4], F32, name="kn2_sb", tag="k2")
                rf_sb = const.tile([SP, 4], F32, name="rf_sb", tag="rf")
                qq_sb = const.tile([SP, 4], F32, name="qq_sb", tag="qq")
                nt_sb = const.tile([SP, 4], F32, name="nt_sb", tag="nt")

                # ---- audio/CIF path, two batches per pass (PE col groups) ----
                prelude_cm = tc.tile_pool(name="preps", bufs=1,
                                          space=bass.MemorySpace.PSUM)
                t_cm = tc.tile_pool(name="tps", bufs=2,
                                    space=bass.MemorySpace.PSUM)
                ds_ps = kw_ps = prelude_cm.__enter__()
                t_ps = t_cm.__enter__()
                for pr in range(B // 2):
                    b0, b1 = 2 * pr, 2 * pr + 1
                    # batch b0 -> PSUM partitions 0-47, b1 -> 64-111
                    dsp0 = ds_ps.tile([128, 384], F32, name="dsp0", tag="dsp0")
                    dsp1 = ds_ps.tile([128, 384], F32, name="dsp1", tag="dsp1")
                    for c in range(TCH):
                        st, sp = c == 0, c == TCH - 1
                        nc.tensor.matmul(dsp0[0:S, :], mt_sb[:, c, b0, :],
                                         at_tiles[b0][:, c, 0:384],
                                         start=st, stop=sp)
                        nc.tensor.matmul(dsp0[64:64 + S, :], mt_sb[:, c, b1, :],
                                         at_tiles[b1][:, c, 0:384],
                                         start=st, stop=sp)
                        nc.tensor.matmul(dsp1[0:S, :], mt_sb[:, c, b0, :],
                                         at_tiles[b0][:, c, 384:768],
                                         start=st, stop=sp)
                        nc.tensor.matmul(dsp1[64:64 + S, :], mt_sb[:, c, b1, :],
                                         at_tiles[b1][:, c, 384:768],
                                         start=st, stop=sp)

                    # pair rows packed [112, A]: rows 0-47 = b0, 64-111 = b1
                    # (rows 48-63 are a hole: zeroed on first slot use, junk
                    # never propagates into kwnT)
                    ds_sb = ds_pool.tile([SP, A], F16, name="ds_sb", tag="ds")
                    if pr < 2:
                        nc.vector.memset(ds_sb[:, :], 0.0)
                    nc.vector.tensor_copy(ds_sb[0:S, 0:384], dsp0[0:S, :])
                    nc.vector.tensor_copy(ds_sb[64:64 + S, 0:384],
                                          dsp0[64:64 + S, :])
                    nc.vector.tensor_copy(ds_sb[0:S, 384:768], dsp1[0:S, :])
                    nc.vector.tensor_copy(ds_sb[64:64 + S, 384:768],
                                          dsp1[64:64 + S, :])

                    dsT_sb = dst_pool.tile([128, AK, SP], F16, name="dsT_sb",
                                           tag="dst")
                    nc.vector.memset(dsT_sb[:, 6, :], 0.0)
                    nc.vector.memset(dsT_sb[0:1, 6, :], 1.0)
                    for k in range(6):
                        tp = t_ps.tile([128, SP], F16, name="tp", tag="tp")
                        nc.tensor.transpose(
                            tp[:], ds_sb[:, k * 128:(k + 1) * 128],
                            ident_sb[:SP, :SP])
                        nc.vector.tensor_copy(dsT_sb[:, k, :], tp[:])

                    kwp = kw_ps.tile([SP, D], F32, name="kwp", tag="kwp")
                    for k in range(AK):
                        nc.tensor.matmul(kwp[:], dsT_sb[:, k, :],
                                         waug_sb[:, k, :],
                                         start=(k == 0), stop=(k == AK - 1))

                    # rf = 10 / max(||kw||, 1e-8)  ==  rsqrt(kn2 * 0.01)
                    sq_sb = sq_pool.tile([SP, D], F16, name="sq_sb", tag="sq")
                    nc.scalar.activation(sq_sb[:], kwp[:], AF.Square,
                                         accum_out=kn2_sb[:, pr:pr + 1])
                    q = qq_sb[:, pr:pr + 1]
                    nc.scalar.mul(q, kn2_sb[:, pr:pr + 1], float(TEMP * TEMP))
                    y = rf_sb[:, pr:pr + 1]
                    nc.scalar.activation(y, q, AF.Sqrt)
                    nc.vector.tensor_scalar_max(y, y, EPS * TEMP)
                    nc.vector.reciprocal(y, y)
                    t1 = nt_sb[:, pr:pr + 1]
                    for _ in range(2):
                        nc.vector.tensor_tensor(t1, y, y, OP.mult)
                        nc.vector.tensor_tensor(t1, t1, q, OP.mult)
                        nc.vector.tensor_scalar(t1, t1, -0.5, 1.5,
                                                OP.mult, OP.add)
                        nc.vector.tensor_tensor(y, y, t1, OP.mult)

                    kwn_sb = kwn_pool.tile([SP, D], F16, name="kwn_sb", tag="kw")
                    nc.vector.tensor_scalar_mul(kwn_sb[:], kwp[:], y)
                    for k in range(4):
                        tp = t_ps.tile([128, SP], F16, name="tp", tag="tp")
                        nc.tensor.transpose(
                            tp[:], kwn_sb[:, k * 128:(k + 1) * 128],
                            ident_sb[:SP, :SP])
                        nc.vector.tensor_copy(
                            kwnT_sb[:, k, pr * S2:pr * S2 + S], tp[:, 0:S])
                        nc.vector.tensor_copy(
                            kwnT_sb[:, k, pr * S2 + S:(pr + 1) * S2],
                            tp[:, 64:64 + S])

                t_cm.__exit__(None, None, None)
                prelude_cm.__exit__(None, None, None)

                # ---- vocab chunk loop: super-chunks of 2 v-chunks ----
                p_cm = tc.tile_pool(name="pps", bufs=2,
                                    space=bass.MemorySpace.PSUM)
                u_cm = tc.tile_pool(name="ups", bufs=1,
                                    space=bass.MemorySpace.PSUM)
                s_cm = tc.tile_pool(name="sps", bufs=1,
                                    space=bass.MemorySpace.PSUM)
                p_ps = p_cm.__enter__()
                u_psp = u_cm.__enter__()
                s_psp = s_cm.__enter__()
                u_tiles = [u_psp.tile([128, D], F32, tag=f"u{i}", name=f"u{i}")
                           for i in range(3)]
                s_tile = s_psp.tile([1, BS], F32, tag="s", name="s")
                nsc = (NCH + 1) // 2
                for sc in range(nsc):
                    chunks = [c for c in (2 * sc, 2 * sc + 1) if c < NCH]
                    # pp: [128, 2, 512] f32 = exactly 2 PSUM banks
                    pp = p_ps.tile([128, 2, D], F32, name="pp", tag="pp")
                    for gi, c in enumerate(chunks):
                        vsz = 128 if c < NCH - 1 else V_TAIL
                        g, off = divmod(c, VGRP)
                        for k in range(4):
                            nc.tensor.matmul(
                                pp[:vsz, gi, 0:BS],
                                ebnt_g[g][:, k, off * 128:off * 128 + vsz],
                                kwnT_sb[:, k, :], start=(k == 0), stop=(k == 3))
                    pt = pt_pool.tile([128, 2, BS], F16, name="pt", tag="pt")
                    if len(chunks) == 2:
                        nc.scalar.activation(pt[:, :, :], pp[:, :, 0:BS], AF.Exp)
                    else:
                        nc.scalar.activation(pt[:V_TAIL, 0, :],
                                             pp[:V_TAIL, 0, 0:BS], AF.Exp)
                    if debug and sc == 0:
                        nc.sync.dma_start(dbg_kwnt_t[:], kwnT_sb[:])
                        nc.sync.dma_start(dbg_pt_t[:], pt[:, 0, :])
                    for gi, c in enumerate(chunks):
                        vsz = 128 if c < NCH - 1 else V_TAIL
                        g, off = divmod(c, VGRP)
                        first = c == 0
                        last = c == NCH - 1
                        for blk in range(3):
                            nc.tensor.matmul(
                                u_tiles[blk][:],
                                pt[:vsz, gi, blk * 128:(blk + 1) * 128],
                                embn_g[g][:vsz, off, :],
                                start=first, stop=last)
                        # s via 1-column stationary ones: negligible LDWEIGHTS
                        nc.tensor.matmul(
                            s_tile[:], ones_sb[:vsz, :], pt[:vsz, gi, :],
                            start=first, stop=last)

                # ---- write out partials ----
                for blk in range(3):
                    us = out_pool.tile([128, D], F32, name="us", tag="us")
                    nc.vector.tensor_copy(us[:], u_tiles[blk][:])
                    nc.sync.dma_start(u_t[blk], us[:])
                ss = out_pool.tile([1, BS], F32, name="ss", tag="ss")
                nc.vector.tensor_copy(ss[:], s_tile[:])
                nc.sync.dma_start(s_t[:], ss[:])

                s_cm.__exit__(None, None, None)
                u_cm.__exit__(None, None, None)
                p_cm.__exit__(None, None, None)

            if repeat == 1:
                body()
            else:
                with tc.For_i(0, repeat, 1):
                    body()

    nc.compile()
    return nc


def _host_prep(audio_feat, W_proj, b_proj, token_emb, fp_alignment):
    """Build the per-core input maps (dtype casts, layout shuffles, CIF alpha)."""
    audio16 = np.ascontiguousarray(
        audio_feat.astype(np.float16)
        .reshape(B, TCH, 128, A).transpose(0, 2, 1, 3))

    # CIF pseudo-alpha matrix M_T[t, b, s] = 1/len_s on frames of segment s.
    fp = np.clip(fp_alignment.astype(np.int64), 0, T)          # [B, S] == cumsum
    lens = np.diff(fp, prepend=0, axis=-1)
    lens = np.clip(lens, 0, None)
    cum = np.cumsum(lens, axis=-1)                             # [B, S]
    start = cum - lens
    tidx = np.arange(T)
    ind = (tidx[None, :, None] >= start[:, None, :]) & \
          (tidx[None, :, None] < cum[:, None, :])              # [B, T, S]
    recip = np.where(lens > 0, 1.0 / np.maximum(lens, 1), 0.0) # [B, S]
    mt = ind * recip[:, None, :]                               # [B, T, S]
    mt16 = np.ascontiguousarray(
        mt.reshape(B, TCH, 128, S).transpose(2, 1, 0, 3).astype(np.float16))

    waug = np.zeros((AK * 128, D), np.float32)
    waug[:A] = W_proj
    waug[A] = b_proj
    waug16 = np.ascontiguousarray(
        waug.reshape(AK, 128, D).transpose(1, 0, 2).astype(np.float16))

    ident16 = np.eye(128, dtype=np.float16)

    en = np.maximum(np.linalg.norm(token_emb, axis=-1), EPS)   # [V]
    ebn = token_emb / en[:, None]                              # [V, D] f32

    shared = {"audio": audio16, "mt": mt16, "waug": waug16, "ident": ident16}
    in_maps = []
    NV = NGRP * VGRP * 128                                     # padded row count
    for i in range(N_CORES):
        v0 = i * VS
        sl_n = np.zeros((NV, D), np.float32)
        sl_n[:VS] = ebn[v0:v0 + VS]
        # ebnt: [128p(of D), NGRP, 4, VGRP*128] ; D index = k*128 + p
        et = sl_n.T.reshape(4, 128, NGRP, VGRP * 128)
        ebnt16 = np.ascontiguousarray(
            et.transpose(1, 2, 0, 3).astype(np.float16))
        raw = np.zeros((NV, D), np.float32)
        raw[:VS] = token_emb[v0:v0 + VS]
        # embn: [128p(of v), NGRP, VGRP, D] ; v = (g*VGRP + j)*128 + p
        embn16 = np.ascontiguousarray(
            raw.reshape(NGRP, VGRP, 128, D).transpose(2, 0, 1, 3)
            .astype(np.float16))
        in_maps.append({**shared, "ebnt": ebnt16, "embn": embn16})
    return in_maps


def kernel(audio_feat, W_proj, b_proj, token_emb, fp_alignment, target_len):
    assert int(target_len) == S
    if "nc" not in _CACHE:
        _CACHE["nc"] = _build_program()
    nc = _CACHE["nc"]

    in_maps = _host_prep(np.asarray(audio_feat, np.float32),
                         np.asarray(W_proj, np.float32),
                         np.asarray(b_proj, np.float32),
                         np.asarray(token_emb, np.float32),
                         np.asarray(fp_alignment))

    res = bass_utils.run_bass_kernel_spmd(nc, in_maps,
                                          core_ids=list(range(N_CORES)))
    u = np.zeros((3, 128, D), np.float64)
    s = np.zeros((1, BS), np.float64)
    for i in range(N_CORES):
        u += res.results[i]["u"]
        s += res.results[i]["s"]
    out = (u.reshape(BS, D) / s.reshape(BS, 1)).astype(np.float32)
    return out.reshape(B, S, D)


# revision 34
# speedup vs baseline: 3.7358x; 2.7051x over previous
"""Trainium2 Bass kernel for nn_CascadedBranch_dynamic (CIF downsample -> proj ->
cosine-vs-token-table -> softmax(VQ) -> re-embed).

Sharding: vocab-parallel over the 8 cores for the heavy matmuls. Each core
holds V/8 = 6176 token-embedding rows in two layouts (D-major normalized for
the cosine-score matmul, V-major raw for the VQ readout) and produces
unnormalized softmax partials u = sum_v exp(logit)*emb[v], s = sum_v
exp(logit); the host combines out = (sum_c u_c) / (sum_c s_c). |logits| <= 10
by construction (cosine / 0.1), so exp never overflows and no max-subtraction
pass is needed anywhere.

The audio/CIF path is batch-parallel: core i CIF-downsamples only batch i
(audio DMA per core drops 12.6 MB -> 1.6 MB) and the tiny per-batch keyword
block [128, 4, 48] fp16 is AllGather-ed across the 8 cores before the vocab
loop. If the collective program fails to build/run, a fully replicated
fallback (every core redoes all 8 batches, no collectives) is used.

Matmuls run in fp16 (inputs cast host-side; PSUM accumulation is fp32).
"""

import sys

for _p in ("/opt/trn_rl_repo", "/root/.axon_site/_ro/trn_rl_repo"):
    if _p not in sys.path:
        sys.path.insert(0, _p)

import numpy as np

import concourse.bacc as bacc
import concourse.bass as bass
import concourse.mybir as mybir
import concourse.tile as tile
from concourse import bass_utils

F16 = mybir.dt.float16
F32 = mybir.dt.float32
AF = mybir.ActivationFunctionType
OP = mybir.AluOpType

B, T, S, A, D, V = 8, 1024, 48, 768, 512, 49408
N_CORES = 8
VS = V // N_CORES            # 6176 table rows per core
NCH = (VS + 127) // 128      # 49 v-chunks (48 full + one of 32)
V_TAIL = VS - 128 * (NCH - 1)  # 32
TCH = T // 128               # 8 time chunks
AK = 7                       # 6 A-chunks of 128 + 1 aug chunk (bias row)
BS = B * S                   # 384 keyword rows
S2 = 2 * S                   # 96 keyword rows per batch pair
SP = 112                     # pair tiles: rows 0-47 = b0, 64-111 = b1
EPS = 1e-8
TEMP = 0.1
VGRP = 7                     # v-chunks per table-DMA group
NGRP = (NCH + VGRP - 1) // VGRP  # 7 groups

_CACHE = {}


def _chunk_loop(nc, tc, pt_pool, out_pool, ebnt_g, embn_g, kwnT_ap, ones_sb,
                u_t, s_t):
    """Vocab loop: scores -> exp -> u/s partial accumulation (shared)."""
    p_cm = tc.tile_pool(name="pps", bufs=2, space=bass.MemorySpace.PSUM)
    u_cm = tc.tile_pool(name="ups", bufs=1, space=bass.MemorySpace.PSUM)
    s_cm = tc.tile_pool(name="sps", bufs=1, space=bass.MemorySpace.PSUM)
    p_ps = p_cm.__enter__()
    u_psp = u_cm.__enter__()
    s_psp = s_cm.__enter__()
    u_tiles = [u_psp.tile([128, D], F32, tag=f"u{i}", name=f"u{i}")
               for i in range(3)]
    s_tile = s_psp.tile([1, BS], F32, tag="s", name="s")
    nsc = (NCH + 1) // 2
    for sc in range(nsc):
        chunks = [c for c in (2 * sc, 2 * sc + 1) if c < NCH]
        pp = p_ps.tile([128, 2, D], F32, name="pp", tag="pp")
        for gi, c in enumerate(chunks):
            vsz = 128 if c < NCH - 1 else V_TAIL
            g, off = divmod(c, VGRP)
            for k in range(4):
                nc.tensor.matmul(
                    pp[:vsz, gi, 0:BS],
                    ebnt_g[g][:, k, off * 128:off * 128 + vsz],
                    kwnT_ap(k), start=(k == 0), stop=(k == 3))
        pt = pt_pool.tile([128, 2, BS], F16, name="pt", tag="pt")
        if len(chunks) == 2:
            nc.scalar.activation(pt[:, :, :], pp[:, :, 0:BS], AF.Exp)
        else:
            nc.scalar.activation(pt[:V_TAIL, 0, :], pp[:V_TAIL, 0, 0:BS],
                                 AF.Exp)
        for gi, c in enumerate(chunks):
            vsz = 128 if c < NCH - 1 else V_TAIL
            g, off = divmod(c, VGRP)
            first, last = c == 0, c == NCH - 1
            for blk in range(3):
                nc.tensor.matmul(
                    u_tiles[blk][:], pt[:vsz, gi, blk * 128:(blk + 1) * 128],
                    embn_g[g][:vsz, off, :], start=first, stop=last)
            # s via 1-column stationary ones: negligible LDWEIGHTS
            nc.tensor.matmul(s_tile[:], ones_sb[:vsz, :], pt[:vsz, gi, :],
                             start=first, stop=last)

    for blk in range(3):
        us = out_pool.tile([128, D], F32, name="us", tag="us")
        nc.vector.tensor_copy(us[:], u_tiles[blk][:])
        nc.sync.dma_start(u_t[blk], us[:])
    ss = out_pool.tile([1, BS], F32, name="ss", tag="ss")
    nc.vector.tensor_copy(ss[:], s_tile[:])
    nc.sync.dma_start(s_t[:], ss[:])

    s_cm.__exit__(None, None, None)
    u_cm.__exit__(None, None, None)
    p_cm.__exit__(None, None, None)


def _norm_chain(nc, kwp, kn2_col, qq_col, rf_col, nt_col):
    """rf = 10 / max(||kw||, 1e-8) == rsqrt(kn2*0.01): coarse Sqrt LUT seed
    + two Newton rsqrt steps (the LUT alone costs ~1e-2 accuracy)."""
    nc.scalar.mul(qq_col, kn2_col, float(TEMP * TEMP))
    nc.scalar.activation(rf_col, qq_col, AF.Sqrt)
    nc.vector.tensor_scalar_max(rf_col, rf_col, EPS * TEMP)
    nc.vector.reciprocal(rf_col, rf_col)
    for _ in range(2):
        nc.vector.tensor_tensor(nt_col, rf_col, rf_col, OP.mult)
        nc.vector.tensor_tensor(nt_col, nt_col, qq_col, OP.mult)
        nc.vector.tensor_scalar(nt_col, nt_col, -0.5, 1.5, OP.mult, OP.add)
        nc.vector.tensor_tensor(rf_col, rf_col, nt_col, OP.mult)


def _build_collective(repeat=1):
    """Batch-parallel CIF prelude + AllGather of kwnT + vocab loop."""
    nc = bacc.Bacc("TRN2", target_bir_lowering=False, debug=False,
                   num_devices=N_CORES)

    audio_t = nc.dram_tensor("audio", [128, TCH, A], F16,
                             kind="ExternalInput").ap()
    mt_t = nc.dram_tensor("mt", [128, TCH, S], F16, kind="ExternalInput").ap()
    waug_t = nc.dram_tensor("waug", [128, AK, D], F16, kind="ExternalInput").ap()
    ident_t = nc.dram_tensor("ident", [128, 128], F16, kind="ExternalInput").ap()
    ebnt_t = nc.dram_tensor("ebnt", [128, NGRP, 4, VGRP * 128], F16,
                            kind="ExternalInput").ap()
    embn_t = nc.dram_tensor("embn", [128, NGRP, VGRP, D], F16,
                            kind="ExternalInput").ap()
    u_t = nc.dram_tensor("u", [3, 128, D], F32, kind="ExternalOutput").ap()
    s_t = nc.dram_tensor("s", [1, BS], F32, kind="ExternalOutput").ap()

    with tile.TileContext(nc) as tc:
        with (
            tc.tile_pool(name="const", bufs=1) as const,
            tc.tile_pool(name="ds", bufs=1) as ds_pool,
            tc.tile_pool(name="dst", bufs=1) as dst_pool,
            tc.tile_pool(name="kwn", bufs=1) as kwn_pool,
            tc.tile_pool(name="sq", bufs=1) as sq_pool,
            tc.tile_pool(name="pt", bufs=4) as pt_pool,
            tc.tile_pool(name="outp", bufs=2) as out_pool,
            tc.tile_pool(name="dram", bufs=1, space="DRAM") as dram_pool,
        ):

            def body():
                mt_sb = const.tile([128, TCH, S], F16, name="mt_sb", tag="mt")
                nc.sync.dma_start(mt_sb[:], mt_t[:])
                waug_sb = const.tile([128, AK, D], F16, name="waug_sb", tag="wa")
                nc.sync.dma_start(waug_sb[:], waug_t[:])
                ident_sb = const.tile([128, 128], F16, name="ident_sb", tag="id")
                nc.sync.dma_start(ident_sb[:], ident_t[:])
                ones_sb = const.tile([128, 1], F16, name="ones_sb", tag="on")
                nc.vector.memset(ones_sb[:], 1.0)
                at = const.tile([128, TCH, A], F16, name="at", tag="at")
                nc.sync.dma_start(at[:], audio_t[:])

                ebnt_g, embn_g = [], []
                for g in range(NGRP):
                    eg = const.tile([128, 4, VGRP * 128], F16,
                                    name=f"ebnt{g}", tag=f"eb{g}")
                    nc.sync.dma_start(eg[:], ebnt_t[:, g])
                    ng = const.tile([128, VGRP, D], F16,
                                    name=f"embn{g}", tag=f"em{g}")
                    nc.sync.dma_start(ng[:], embn_t[:, g])
                    ebnt_g.append(eg)
                    embn_g.append(ng)

                kwnT_sb = const.tile([128, 4, B, S], F16, name="kwnT_sb",
                                     tag="kt")
                kn2_sb = const.tile([S, 1], F32, name="kn2_sb", tag="k2")
                rf_sb = const.tile([S, 1], F32, name="rf_sb", tag="rf")
                qq_sb = const.tile([S, 1], F32, name="qq_sb", tag="qq")
                nt_sb = const.tile([S, 1], F32, name="nt_sb", tag="nt")
                kwt_sb = const.tile([128, 4, S], F16, name="kwt_sb", tag="kl")

                prelude_cm = tc.tile_pool(name="preps", bufs=1,
                                          space=bass.MemorySpace.PSUM)
                t_cm = tc.tile_pool(name="tps", bufs=2,
                                    space=bass.MemorySpace.PSUM)
                ds_ps = kw_ps = prelude_cm.__enter__()
                t_ps = t_cm.__enter__()

                dsp0 = ds_ps.tile([S, 384], F32, name="dsp0", tag="dsp0")
                dsp1 = ds_ps.tile([S, 384], F32, name="dsp1", tag="dsp1")
                for c in range(TCH):
                    st, sp = c == 0, c == TCH - 1
                    nc.tensor.matmul(dsp0[:], mt_sb[:, c, :], at[:, c, 0:384],
                                     start=st, stop=sp)
                    nc.tensor.matmul(dsp1[:], mt_sb[:, c, :], at[:, c, 384:768],
                                     start=st, stop=sp)

                ds_sb = ds_pool.tile([S, A], F16, name="ds_sb", tag="ds")
                nc.vector.tensor_copy(ds_sb[:, 0:384], dsp0[:])
                nc.vector.tensor_copy(ds_sb[:, 384:768], dsp1[:])

                dsT_sb = dst_pool.tile([128, AK, S], F16, name="dsT_sb",
                                       tag="dst")
                nc.vector.memset(dsT_sb[:, 6, :], 0.0)
                nc.vector.memset(dsT_sb[0:1, 6, :], 1.0)
                for k in range(6):
                    tp = t_ps.tile([128, S], F16, name="tp", tag="tp")
                    nc.tensor.transpose(tp[:], ds_sb[:, k * 128:(k + 1) * 128],
                                        ident_sb[:S, :S])
                    nc.vector.tensor_copy(dsT_sb[:, k, :], tp[:])

                kwp = kw_ps.tile([S, D], F32, name="kwp", tag="kwp")
                for k in range(AK):
                    nc.tensor.matmul(kwp[:], dsT_sb[:, k, :], waug_sb[:, k, :],
                                     start=(k == 0), stop=(k == AK - 1))

                sq_sb = sq_pool.tile([S, D], F16, name="sq_sb", tag="sq")
                nc.scalar.activation(sq_sb[:], kwp[:], AF.Square,
                                     accum_out=kn2_sb[:, 0:1])
                _norm_chain(nc, kwp, kn2_sb[:, 0:1], qq_sb[:, 0:1],
                            rf_sb[:, 0:1], nt_sb[:, 0:1])

                kwn_sb = kwn_pool.tile([S, D], F16, name="kwn_sb", tag="kw")
                nc.vector.tensor_scalar_mul(kwn_sb[:], kwp[:], rf_sb[:, 0:1])
                for k in range(4):
                    tp = t_ps.tile([128, S], F16, name="tp", tag="tp")
                    nc.tensor.transpose(tp[:], kwn_sb[:, k * 128:(k + 1) * 128],
                                        ident_sb[:S, :S])
                    nc.vector.tensor_copy(kwt_sb[:, k, :], tp[:])

                # all-gather the per-batch keyword blocks (48 KB each)
                cc_in = dram_pool.tile([128, 4 * S], F16, name="cc_in", tag="ci")
                cc_out = dram_pool.tile([B, 128, 4 * S], F16, name="cc_out",
                                        tag="co", addr_space="Shared")
                nc.sync.dma_start(cc_in[:], kwt_sb[:])
                nc.gpsimd.collective_compute(
                    "AllGather", OP.bypass,
                    replica_groups=[list(range(N_CORES))],
                    ins=[cc_in.opt()], outs=[cc_out.opt()])
                nc.sync.dma_start(
                    kwnT_sb[:], cc_out.rearrange("b p (k s) -> p k b s", k=4))

                t_cm.__exit__(None, None, None)
                prelude_cm.__exit__(None, None, None)

                _chunk_loop(nc, tc, pt_pool, out_pool, ebnt_g, embn_g,
                            lambda k: kwnT_sb[:, k, :, :], ones_sb, u_t, s_t)

            # collectives cannot sit inside a hardware loop -> static unroll
            for _ in range(repeat):
                body()

    nc.compile()
    return nc


def _build_replicated(repeat=1, loop=True):
    """Fallback: no collectives; every core redoes the full audio path with
    batch pairs packed into disjoint PE column groups."""
    nc = bacc.Bacc("TRN2", target_bir_lowering=False, debug=False,
                   num_devices=N_CORES)

    audio_t = nc.dram_tensor("audio", [B, 128, TCH, A], F16,
                             kind="ExternalInput").ap()
    mt_t = nc.dram_tensor("mt", [128, TCH, B, S], F16, kind="ExternalInput").ap()
    waug_t = nc.dram_tensor("waug", [128, AK, D], F16, kind="ExternalInput").ap()
    ident_t = nc.dram_tensor("ident", [128, 128], F16, kind="ExternalInput").ap()
    ebnt_t = nc.dram_tensor("ebnt", [128, NGRP, 4, VGRP * 128], F16,
                            kind="ExternalInput").ap()
    embn_t = nc.dram_tensor("embn", [128, NGRP, VGRP, D], F16,
                            kind="ExternalInput").ap()
    u_t = nc.dram_tensor("u", [3, 128, D], F32, kind="ExternalOutput").ap()
    s_t = nc.dram_tensor("s", [1, BS], F32, kind="ExternalOutput").ap()

    with tile.TileContext(nc) as tc:
        with (
            tc.tile_pool(name="const", bufs=1) as const,
            tc.tile_pool(name="audio", bufs=3) as audio_pool,
            tc.tile_pool(name="ds", bufs=2) as ds_pool,
            tc.tile_pool(name="dst", bufs=2) as dst_pool,
            tc.tile_pool(name="kwn", bufs=2) as kwn_pool,
            tc.tile_pool(name="sq", bufs=2) as sq_pool,
            tc.tile_pool(name="pt", bufs=4) as pt_pool,
            tc.tile_pool(name="outp", bufs=2) as out_pool,
        ):

            def body():
                mt_sb = const.tile([128, TCH, B, S], F16, name="mt_sb", tag="mt")
                nc.sync.dma_start(mt_sb[:], mt_t[:])
                waug_sb = const.tile([128, AK, D], F16, name="waug_sb", tag="wa")
                nc.sync.dma_start(waug_sb[:], waug_t[:])
                ident_sb = const.tile([128, 128], F16, name="ident_sb", tag="id")
                nc.sync.dma_start(ident_sb[:], ident_t[:])
                ones_sb = const.tile([128, 1], F16, name="ones_sb", tag="on")
                nc.vector.memset(ones_sb[:], 1.0)

                at_tiles = []
                for b in range(B):
                    at = audio_pool.tile([128, TCH, A], F16, name="at", tag="at")
                    nc.sync.dma_start(at[:], audio_t[b])
                    at_tiles.append(at)

                ebnt_g, embn_g = [], []
                for g in range(NGRP):
                    eg = const.tile([128, 4, VGRP * 128], F16,
                                    name=f"ebnt{g}", tag=f"eb{g}")
                    nc.sync.dma_start(eg[:], ebnt_t[:, g])
                    ng = const.tile([128, VGRP, D], F16,
                                    name=f"embn{g}", tag=f"em{g}")
                    nc.sync.dma_start(ng[:], embn_t[:, g])
                    ebnt_g.append(eg)
                    embn_g.append(ng)

                kwnT_sb = const.tile([128, 4, BS], F16, name="kwnT_sb", tag="kt")
                kn2_sb = const.tile([SP, 4], F32, name="kn2_sb", tag="k2")
                rf_sb = const.tile([SP, 4], F32, name="rf_sb", tag="rf")
                qq_sb = const.tile([SP, 4], F32, name="qq_sb", tag="qq")
                nt_sb = const.tile([SP, 4], F32, name="nt_sb", tag="nt")

                prelude_cm = tc.tile_pool(name="preps", bufs=1,
                                          space=bass.MemorySpace.PSUM)
                t_cm = tc.tile_pool(name="tps", bufs=2,
                                    space=bass.MemorySpace.PSUM)
                ds_ps = kw_ps = prelude_cm.__enter__()
                t_ps = t_cm.__enter__()
                for pr in range(B // 2):
                    b0, b1 = 2 * pr, 2 * pr + 1
                    dsp0 = ds_ps.tile([128, 384], F32, name="dsp0", tag="dsp0")
                    dsp1 = ds_ps.tile([128, 384], F32, name="dsp1", tag="dsp1")
                    for c in range(TCH):
                        st, sp = c == 0, c == TCH - 1
                        nc.tensor.matmul(dsp0[0:S, :], mt_sb[:, c, b0, :],
                                         at_tiles[b0][:, c, 0:384],
                                         start=st, stop=sp)
                        nc.tensor.matmul(dsp0[64:64 + S, :], mt_sb[:, c, b1, :],
                                         at_tiles[b1][:, c, 0:384],
                                         start=st, stop=sp)
                        nc.tensor.matmul(dsp1[0:S, :], mt_sb[:, c, b0, :],
                                         at_tiles[b0][:, c, 384:768],
                                         start=st, stop=sp)
                        nc.tensor.matmul(dsp1[64:64 + S, :], mt_sb[:, c, b1, :],
                                         at_tiles[b1][:, c, 384:768],
                                         start=st, stop=sp)

                    ds_sb = ds_pool.tile([SP, A], F16, name="ds_sb", tag="ds")
                    if pr < 2:
                        nc.vector.memset(ds_sb[:, :], 0.0)
                    nc.vector.tensor_copy(ds_sb[0:S, 0:384], dsp0[0:S, :])
                    nc.vector.tensor_copy(ds_sb[64:64 + S, 0:384],
                                          dsp0[64:64 + S, :])
                    nc.vector.tensor_copy(ds_sb[0:S, 384:768], dsp1[0:S, :])
                    nc.vector.tensor_copy(ds_sb[64:64 + S, 384:768],
                                          dsp1[64:64 + S, :])

                    dsT_sb = dst_pool.tile([128, AK, SP], F16, name="dsT_sb",
                                           tag="dst")
                    nc.vector.memset(dsT_sb[:, 6, :], 0.0)
                    nc.vector.memset(dsT_sb[0:1, 6, :], 1.0)
                    for k in range(6):
                        tp = t_ps.tile([128, SP], F16, name="tp", tag="tp")
                        nc.tensor.transpose(
                            tp[:], ds_sb[:, k * 128:(k + 1) * 128],
                            ident_sb[:SP, :SP])
                        nc.vector.tensor_copy(dsT_sb[:, k, :], tp[:])

                    kwp = kw_ps.tile([SP, D], F32, name="kwp", tag="kwp")
                    for k in range(AK):
                        nc.tensor.matmul(kwp[:], dsT_sb[:, k, :],
                                         waug_sb[:, k, :],
                                         start=(k == 0), stop=(k == AK - 1))

                    sq_sb = sq_pool.tile([SP, D], F16, name="sq_sb", tag="sq")
                    nc.scalar.activation(sq_sb[:], kwp[:], AF.Square,
                                         accum_out=kn2_sb[:, pr:pr + 1])
                    _norm_chain(nc, kwp, kn2_sb[:, pr:pr + 1],
                                qq_sb[:, pr:pr + 1], rf_sb[:, pr:pr + 1],
                                nt_sb[:, pr:pr + 1])

                    kwn_sb = kwn_pool.tile([SP, D], F16, name="kwn_sb", tag="kw")
                    nc.vector.tensor_scalar_mul(kwn_sb[:], kwp[:],
                                                rf_sb[:, pr:pr + 1])
                    for k in range(4):
                        tp = t_ps.tile([128, SP], F16, name="tp", tag="tp")
                        nc.tensor.transpose(
                            tp[:], kwn_sb[:, k * 128:(k + 1) * 128],
                            ident_sb[:SP, :SP])
                        nc.vector.tensor_copy(
                            kwnT_sb[:, k, pr * S2:pr * S2 + S], tp[:, 0:S])
                        nc.vector.tensor_copy(
                            kwnT_sb[:, k, pr * S2 + S:(pr + 1) * S2],
                            tp[:, 64:64 + S])

                t_cm.__exit__(None, None, None)
                prelude_cm.__exit__(None, None, None)

                _chunk_loop(nc, tc, pt_pool, out_pool, ebnt_g, embn_g,
                            lambda k: kwnT_sb[:, k, :], ones_sb, u_t, s_t)

            if repeat == 1:
                body()
            elif loop:
                with tc.For_i(0, repeat, 1):
                    body()
            else:
                for _ in range(repeat):
                    body()

    nc.compile()
    return nc


def _host_prep(audio_feat, W_proj, b_proj, token_emb, fp_alignment,
               collective=True):
    """Build the per-core input maps (dtype casts, layout shuffles, CIF alpha)."""
    audio16 = np.ascontiguousarray(
        audio_feat.astype(np.float16)
        .reshape(B, TCH, 128, A).transpose(0, 2, 1, 3))  # [B, 128, TCH, A]

    # CIF pseudo-alpha matrix M_T[t, b, s] = 1/len_s on frames of segment s
    # (mirrors the reference's clip/diff/cumsum semantics exactly).
    fp = np.clip(fp_alignment.astype(np.int64), 0, T)
    lens = np.clip(np.diff(fp, prepend=0, axis=-1), 0, None)
    cum = np.cumsum(lens, axis=-1)
    start = cum - lens
    tidx = np.arange(T)
    ind = (tidx[None, :, None] >= start[:, None, :]) & \
          (tidx[None, :, None] < cum[:, None, :])              # [B, T, S]
    recip = np.where(lens > 0, 1.0 / np.maximum(lens, 1), 0.0)
    mt = ind * recip[:, None, :]                               # [B, T, S]
    mt16 = np.ascontiguousarray(
        mt.reshape(B, TCH, 128, S).transpose(2, 1, 0, 3).astype(np.float16))

    waug = np.zeros((AK * 128, D), np.float32)
    waug[:A] = W_proj
    waug[A] = b_proj
    waug16 = np.ascontiguousarray(
        waug.reshape(AK, 128, D).transpose(1, 0, 2).astype(np.float16))

    ident16 = np.eye(128, dtype=np.float16)

    en = np.maximum(np.linalg.norm(token_emb, axis=-1), EPS)
    ebn = token_emb / en[:, None]                              # [V, D] f32

    shared = {"waug": waug16, "ident": ident16}
    in_maps = []
    NV = NGRP * VGRP * 128
    for i in range(N_CORES):
        v0 = i * VS
        sl_n = np.zeros((NV, D), np.float32)
        sl_n[:VS] = ebn[v0:v0 + VS]
        et = sl_n.T.reshape(4, 128, NGRP, VGRP * 128)
        ebnt16 = np.ascontiguousarray(
            et.transpose(1, 2, 0, 3).astype(np.float16))
        raw = np.zeros((NV, D), np.float32)
        raw[:VS] = token_emb[v0:v0 + VS]
        embn16 = np.ascontiguousarray(
            raw.reshape(NGRP, VGRP, 128, D).transpose(2, 0, 1, 3)
            .astype(np.float16))
        m = {**shared, "ebnt": ebnt16, "embn": embn16}
        if collective:
            m["audio"] = audio16[i]                            # own batch only
            m["mt"] = np.ascontiguousarray(mt16[:, :, i, :])
        else:
            m["audio"] = audio16
            m["mt"] = mt16
        in_maps.append(m)
    return in_maps


def _get_program():
    if "mode" not in _CACHE:
        try:
            _CACHE["nc"] = _build_collective()
            _CACHE["mode"] = "collective"
        except Exception:
            _CACHE["nc"] = _build_replicated()
            _CACHE["mode"] = "replicated"
    return _CACHE["nc"], _CACHE["mode"]


def kernel(audio_feat, W_proj, b_proj, token_emb, fp_alignment, target_len):
    assert int(target_len) == S
    nc, mode = _get_program()
    args = (np.asarray(audio_feat, np.float32), np.asarray(W_proj, np.float32),
            np.asarray(b_proj, np.float32), np.asarray(token_emb, np.float32),
            np.asarray(fp_alignment))
    in_maps = _host_prep(*args, collective=(mode == "collective"))
    try:
        res = bass_utils.run_bass_kernel_spmd(nc, in_maps,
                                              core_ids=list(range(N_CORES)))
    except Exception:
        if mode != "collective":
            raise
        # fall back to the collective-free program once
        _CACHE["nc"] = _build_replicated()
        _CACHE["mode"] = "replicated"
        nc = _CACHE["nc"]
        in_maps = _host_prep(*args, collective=False)
        res = bass_utils.run_bass_kernel_spmd(nc, in_maps,
                                              core_ids=list(range(N_CORES)))

    u = np.zeros((3, 128, D), np.float64)
    s = np.zeros((1, BS), np.float64)
    for i in range(N_CORES):
        u += res.results[i]["u"]
        s += res.results[i]["s"]
    out = (u.reshape(BS, D) / s.reshape(BS, 1)).astype(np.float32)
    return out.reshape(B, S, D)
